# revision 76
# baseline (speedup 1.0000x reference)
"""CropToBBox (crop_and_resize to 224x224 with bbox preprocessing) on 8 trn2 cores.

v2 strategy (vs full-image fp32 baseline):
  - Gather-DMA only the needed source window per image: rows [r0, r0+S) and a
    64px-aligned column window of K blocks, via SWDGE dma_gather with
    host-computed int16 indices (idx = 24*h + 3*cb0 in 256B units).
  - Host assigns images to 8 (core, slot) pairs so each slot has similar
    window sizes; per-slot (S, K) are compile-time (kernel is rebuilt if the
    size signature changes), indices/coords stay runtime inputs.
  - Separable bilinear resize as two matmul stages in float32r with 256-wide
    moving dims (full-rate on the PE vs 1/4 for plain fp32).
  - hat weights built on device from gpsimd-broadcast coords; negated-hat
    variant on DVE/gpsimd (sign cancels across the two stages), positive
    variant on ACT. Copies greedily load-balanced across DVE/ACT/gpsimd.
  - Output written planar [n, c, i, j]; host does the final NHWC transpose.
"""

import numpy as np

N_FULL = 64
H = W = 512
C = 3
CH = CW = 224
NPAD = 256           # padded free dim for fp32r full-rate matmuls
N_CORES = 8
PER_CORE = N_FULL // N_CORES
FACTOR = 1.2
BAD = np.float32(-1e5)

_CACHE = {}


def _host_coords(threshold, bboxes):
    """Replicate process_bbox + crop_and_resize coordinate math in fp32.

    Returns ys, xs [64, 224] with BAD at invalid (out-of-range) positions.
    """
    f = np.float32
    th = np.asarray(threshold, f)
    bb = np.asarray(bboxes, f)
    default = np.array([0.0, 1.0, 0.0, 1.0], f)
    filt = np.where(th < f(0.5), default, bb).astype(f)
    x1, y1, x2, y2 = filt[:, 0], filt[:, 1], filt[:, 2], filt[:, 3]

    def resize_side(small, large):
        side = (large - small).astype(f)
        new_side = (side * f(FACTOR)).astype(f)
        center = ((small + large) / f(2)).astype(f)
        half = (new_side / f(2)).astype(f)
        new_min = np.clip((center - half).astype(f), f(0), f(1)).astype(f)
        new_max = np.clip((center + half).astype(f), f(0), f(1)).astype(f)
        return new_min, new_max

    nx1, nx2 = resize_side(x1, x2)
    ny1, ny2 = resize_side(y1, y2)
    # reference: boxes = stack([nx1, ny1, nx2, ny2]); crop uses [y1,x1,y2,x2]
    by1, bx1, by2, bx2 = nx1, ny1, nx2, ny2

    idx = np.arange(CH, dtype=f)
    ys = (by1[:, None] * f(H - 1)).astype(f) + (
        idx[None, :] * (((by2 - by1) * f(H - 1)).astype(f) / f(CH - 1)).astype(f)[:, None]
    ).astype(f)
    ys = ys.astype(f)
    xs = (bx1[:, None] * f(W - 1)).astype(f) + (
        idx[None, :] * (((bx2 - bx1) * f(W - 1)).astype(f) / f(CW - 1)).astype(f)[:, None]
    ).astype(f)
    xs = xs.astype(f)

    ys = np.where((ys >= f(0)) & (ys <= f(H - 1)), ys, BAD).astype(f)
    xs = np.where((xs >= f(0)) & (xs <= f(W - 1)), xs, BAD).astype(f)
    return ys, xs


def _windows(ys, xs):
    """Per image: row window (r0, S) and 64px col-block window (cb0, K)."""
    out = []
    for n in range(N_FULL):
        yv = ys[n][ys[n] > -1e4]
        xv = xs[n][xs[n] > -1e4]
        if yv.size == 0 or xv.size == 0:
            out.append((0, 1, 0, 1))
            continue
        r0 = int(np.floor(yv.min())); r1 = int(np.ceil(yv.max()))
        r0 = max(0, min(r0, H - 1)); r1 = max(r0, min(r1, H - 1))
        c0 = int(np.floor(xv.min())); c1 = int(np.ceil(xv.max()))
        c0 = max(0, min(c0, W - 1)); c1 = max(c0, min(c1, W - 1))
        cb0 = c0 // 64
        K = c1 // 64 - cb0 + 1
        out.append((r0, r1 - r0 + 1, cb0, K))
    return out


def _slot_cost(S, K):
    """Rough per-slot ns cost: gather DMA + PE + vector-engine work."""
    Wpx = 64 * K
    Hc = -(-S // 128)
    Wc = -(-Wpx // 128)
    elem_b = K * 768
    per_desc = max(elem_b * (2.0 if elem_b < 512 else 1.0) / 22.5, 7.0)
    dma = S / 16.0 * per_desc                      # gather transfer (shared DMA)
    pe = (3 * Wc * Hc + 6 * Wc) * 107.0           # matmuls
    wts = (Hc + Wc) * (256 * 2 * 1.04 + 394.0)    # weight elems + bc DMA
    cps = Wc * 950.0                              # stage-1 copies
    return 0.8 * dma + 0.8 * pe + 0.5 * (wts + cps)


def _plan(wins):
    """Assign 64 images to 8 slots x 8 cores; returns perm and signature."""
    area = np.array([w[1] * w[3] for w in wins])
    order = np.argsort(-area, kind="stable")
    groups = [list(order[j * 8:(j + 1) * 8]) for j in range(8)]

    def gcost(g):
        S = max(wins[i][1] for i in g)
        K = max(wins[i][3] for i in g)
        return _slot_cost(S, K)

    for _ in range(6):
        improved = False
        for a in range(8):
            for b in range(a + 1, 8):
                base = gcost(groups[a]) + gcost(groups[b])
                best = None
                for ia in range(8):
                    for ib in range(8):
                        ga = groups[a][:]; gb = groups[b][:]
                        ga[ia], gb[ib] = gb[ib], ga[ia]
                        c = gcost(ga) + gcost(gb)
                        if c < base - 1e-9:
                            base = c; best = (ia, ib)
                if best is not None:
                    ia, ib = best
                    groups[a][ia], groups[b][ib] = groups[b][ib], groups[a][ia]
                    improved = True
        if not improved:
            break

    groups.sort(key=gcost)  # program order: small slots first
    sig = []
    perm = [[0] * 8 for _ in range(N_CORES)]
    for j, g in enumerate(groups):
        S = max(wins[i][1] for i in g)
        K = max(wins[i][3] for i in g)
        sig.append((S, K))
        for c, img in enumerate(g):
            perm[c][j] = img
    return perm, tuple(sig)


def _build_nc(sig):
    from concourse import bacc, tile
    from concourse import library_config
    import concourse.mybir as mybir
    import bass_rust

    dt = mybir.dt
    F32 = dt.float32
    F32R = dt.float32r
    I16 = dt.int16
    AF = mybir.ActivationFunctionType
    ALU = mybir.AluOpType

    slots = []
    idx_off = 0
    for S, K in sig:
        Hc = -(-S // 128)
        Wc = -(-(64 * K) // 128)
        cols = -(-S // 16)
        slots.append(dict(S=S, K=K, Hc=Hc, Wc=Wc, icols=cols, ioff=idx_off))
        idx_off += cols
    TOTC = idx_off

    nc = bacc.Bacc(None, target_bir_lowering=False, num_swdge_queues=4)
    images_d = nc.declare_dram_parameter("images", [PER_CORE, H, W, C], F32R, isOutput=False)
    meta_d = nc.declare_dram_parameter("meta", [8 * 2 * NPAD, 32], F32, isOutput=False)
    idxs_d = nc.declare_dram_parameter("idxs", [128, TOTC], I16, isOutput=False)
    wneg_d = nc.declare_dram_parameter("wneg", [128, 4], F32, isOutput=False)
    out_d = nc.declare_dram_parameter("out", [PER_CORE, C, CH, CW], F32, isOutput=True)

    # greedy engine load balance (ns estimates); gpsimd pre-charged with
    # gather desc-gen; sync participates only in DMA issue
    load = {"vector": 0.0, "scalar": 0.0, "gpsimd": 0.0, "sync": 0.0}
    load["gpsimd"] += sum(994 + 0.34 * s["S"] for s in slots)

    def op_cost(eng, free):
        if eng == "vector":
            return free * 1.04 + 160.0
        if eng == "scalar":
            return free * 0.833 + 370.0
        return free * 1.39 + 130.0  # gpsimd

    def pick_copy(free):
        # psum-reading ops: gpsimd cannot access PSUM
        cands = ["vector", "scalar"]
        name = min(cands, key=lambda e: load[e] + op_cost(e, free))
        load[name] += op_cost(name, free)
        return name

    def pick_dma(bytes_per_partition):
        # HWDGE dma cost: ~0.385 ns per byte of partition line on the engine
        cost = bytes_per_partition * 0.385 + 150.0
        cands = ["sync", "scalar"]
        name = min(cands, key=lambda e: load[e] + cost)
        load[name] += cost
        return name

    with tile.TileContext(nc) as tc:
        with (
            tc.tile_pool(name="const", bufs=1) as cpool,
            tc.tile_pool(name="img", bufs=1) as ipool,
            tc.tile_pool(name="bc", bufs=2) as bcpool,
            tc.tile_pool(name="wts", bufs=3) as wpool,
            tc.tile_pool(name="tmp", bufs=2) as tpool,
            tc.tile_pool(name="vt", bufs=3) as vpool,
            tc.tile_pool(name="outsb", bufs=2) as opool,
            tc.tile_pool(name="ps1", bufs=3, space="PSUM") as ps1_pool,
            tc.tile_pool(name="ps1b", bufs=1, space="PSUM") as ps1b_pool,
            tc.tile_pool(name="ps2", bufs=2, space="PSUM") as ps2_pool,
            tc.tile_pool(name="psbc", bufs=2, space="PSUM") as psbc_pool,
        ):
            nc.gpsimd.load_library(library_config.mlp)

            idx_sb = cpool.tile([128, TOTC], I16)
            nc.sync.dma_start(out=idx_sb[:], in_=idxs_d[:])
            wneg = cpool.tile([128, 4], F32)
            nc.sync.dma_start(out=wneg[:], in_=wneg_d[:])
            # per-slot XBAR transpose of a [512, 32] slab whose column 0
            # holds the slot's coords: row 0 of the output lands on
            # partition 0, ready for gpsimd partition_broadcast.
            sbufTs = []
            for j in range(8):
                t = cpool.tile([32, 2 * NPAD], F32, name=f"mT{j}")
                teng = nc.sync if j < 4 else nc.scalar
                teng.dma_start_transpose(
                    t[:], meta_d[j * 2 * NPAD:(j + 1) * 2 * NPAD, :])
                sbufTs.append(t)
            # all-ones lhsT: slab rows 1..31 are zero, so a full-ones
            # selector still extracts row 0 exactly
            ones32 = cpool.tile([32, 128], F32)
            nc.vector.memset(ones32[:], 1.0)

            def copy_op(dst, src, free):
                e = pick_copy(free)
                if e == "scalar":
                    nc.scalar.activation(dst, src, AF.Copy, bias=0.0, scale=1.0)
                else:
                    getattr(nc, e).tensor_copy(dst, src)

            # issue all gathers up front, biggest transfers first so the
            # large slots' compute can start as early as possible
            ximg = [None] * len(slots)
            gq = [0]  # sequential pool-DMA counter: queue must track the
                      # tile framework's round-robin DMASW lane assignment

            def gorder(q):
                a = slots[q]["S"] * slots[q]["K"]
                return a if a <= 128 else 10 ** 9 - a

            for j in sorted(range(len(slots)), key=gorder):
                s = slots[j]
                S, K, Hc = s["S"], s["K"], s["Hc"]
                elem = K * 192
                xt = ipool.tile([128, Hc, elem], F32R, name=f"X{j}", tag=f"X{j}")
                nrow = 12289 - 3 * K
                in_ap = bass_rust.AP(
                    tensor=images_d, offset=j * (H * W * C),
                    ap=[[64, nrow], [1, elem]],
                )
                nc.gpsimd.dma_gather(
                    xt[:], in_ap, idx_sb[:, s["ioff"]:s["ioff"] + s["icols"]],
                    S, S, elem, elem_step=64, queue_num=gq[0] % 4,
                )
                gq[0] += 1
                ximg[j] = xt

            pend = []
            for j, s in enumerate(slots):
                S, K, Hc, Wc = s["S"], s["K"], s["Hc"], s["Wc"]
                Wpx = 64 * K

                # broadcast slot coords [1, 512] -> [128, 512]: exact fp32
                # matmul while PE has slack (early slots), gpsimd
                # partition_broadcast once PE saturates (late slots)
                if j < 5:
                    bct = psbc_pool.tile([128, 2 * NPAD], F32, tag="bc")
                    bc = bct[:]
                    nc.tensor.matmul(
                        bc, ones32[:], sbufTs[j][0:32, :],
                        start=True, stop=True,
                    )
                else:
                    bct = bcpool.tile([128, 2 * NPAD], F32, tag="bc")
                    bc = bct[:]
                    nc.gpsimd.partition_broadcast(bc, sbufTs[j][0:1, :])
                    load["gpsimd"] += 450.0

                # weights: u-ops read PSUM (DVE/ACT only); the nh-op engine
                # decides the slot's sign — scalar builds +hat, vector and
                # gpsimd build -hat (sign cancels across the two stages)
                nchunks = Hc + Wc
                cands = ["vector", "scalar", "gpsimd"]
                weng = min(cands, key=lambda e: load[e] + nchunks * op_cost(e, NPAD) * (0.8 if e == "gpsimd" else 1.0))
                load[weng] += nchunks * op_cost(weng, NPAD)

                nht = wpool.tile([128, 8 * NPAD], F32R, tag="nh")
                wtiles = [nht[:, k * NPAD:(k + 1) * NPAD] for k in range(nchunks)]
                for k in range(nchunks):
                    if k < Hc:
                        src = bc[:, 0:NPAD]
                        kidx = k
                        rows = min(128, S - 128 * k)
                    else:
                        kidx = k - Hc
                        src = bc[:, NPAD:2 * NPAD]
                        rows = min(128, Wpx - 128 * kidx)
                    wt = wtiles[k]
                    u = tpool.tile([128, NPAD], F32, tag="u")
                    # u-op: only ACT has an Abs (abs_max is not valid ISA on
                    # the DVE tensor_scalar path)
                    nc.scalar.activation(
                        u[0:rows, :], src[0:rows, :], AF.Abs,
                        bias=wneg[0:rows, kidx:kidx + 1], scale=1.0,
                    )
                    load["scalar"] += op_cost("scalar", NPAD)
                    if weng == "scalar":
                        nc.scalar.activation(
                            wt[0:rows, :], u[0:rows, :], AF.Relu,
                            bias=1.0, scale=-1.0,
                        )
                    else:
                        getattr(nc, weng).tensor_scalar(
                            out=wt[0:rows, :], in0=u[0:rows, :],
                            scalar1=1.0, scalar2=0.0,
                            op0=ALU.subtract, op1=ALU.min,
                        )

                xv = ximg[j][:].rearrange("p hc (w c) -> p hc w c", c=C)

                # stage 1: V[w, i] = sum_h img[h, w, c] * ryt[h, i]
                vts = []  # per wk: (v2 [128, 448] = ci0|ci1, v1 [128, 224] = ci2)
                for wk in range(Wc):
                    wseg = min(128, Wpx - 128 * wk)
                    pv2 = ps1_pool.tile([128, 2 * NPAD], F32, tag="pv2")
                    pv1 = ps1b_pool.tile([128, NPAD], F32, tag="pv1")
                    for ci in range(C):
                        dst = (pv2[0:wseg, ci * NPAD:(ci + 1) * NPAD] if ci < 2
                               else pv1[0:wseg, :])
                        for k in range(Hc):
                            rows = min(128, S - 128 * k)
                            nc.tensor.matmul(
                                dst,
                                xv[0:rows, k, 128 * wk:128 * wk + wseg, ci],
                                wtiles[k][0:rows, :],
                                start=(k == 0),
                                stop=(k == Hc - 1),
                            )
                    v2 = vpool.tile([128, 2 * CH], F32R, tag=f"v2_{wk}")
                    v1 = vpool.tile([128, CH], F32R, tag=f"v1_{wk}")
                    src2 = pv2[0:wseg, :].rearrange("p (c i) -> p c i", c=2)[:, :, 0:CH]
                    dst2 = v2[0:wseg, :].rearrange("p (c i) -> p c i", c=2)
                    copy_op(dst2, src2, 2 * CH)
                    copy_op(v1[0:wseg, :], pv1[0:wseg, 0:CH], CH)
                    vts.append((v2, v1))

                # stage 2 emission is deferred one slot: emitting it after
                # the NEXT slot's stage 1 gives the in-order PE queue ready
                # matmuls to run where stage 2 would stall on the v copies
                def emit_stage2(j, vts, wtiles, Hc, Wc, Wpx):
                    for ci in range(C):
                        po = ps2_pool.tile([112, 2 * NPAD], F32, tag="po")
                        for ic in range(2):
                            dst = po[:, ic * NPAD:(ic + 1) * NPAD]
                            for wk in range(Wc):
                                wseg = min(128, Wpx - 128 * wk)
                                v2, v1 = vts[wk]
                                if ci < 2:
                                    lhs = v2[0:wseg, ci * CH + ic * 112: ci * CH + ic * 112 + 112]
                                else:
                                    lhs = v1[0:wseg, ic * 112:ic * 112 + 112]
                                nc.tensor.matmul(
                                    dst,
                                    lhs,
                                    wtiles[Hc + wk][0:wseg, :],
                                    start=(wk == 0),
                                    stop=(wk == Wc - 1),
                                )
                        osb = opool.tile([112, 2 * CW], F32, tag=f"osb{ci}")
                        srco = po[:, :].rearrange("p (a jj) -> p a jj", a=2)[:, :, 0:CW]
                        dsto = osb[:].rearrange("p (a jj) -> p a jj", a=2)
                        copy_op(dsto, srco, 2 * CW)
                        e = pick_dma(2 * CW * 4)
                        getattr(nc, e).dma_start(
                            out=out_d[j, ci].rearrange("(ic p) jj -> p ic jj", ic=2),
                            in_=osb[:].rearrange("p (ic jj) -> p ic jj", ic=2),
                        )

                pend.append((j, vts, wtiles, Hc, Wc, Wpx))
                if len(pend) > 1:
                    emit_stage2(*pend.pop(0))
            for p in pend:
                emit_stage2(*p)
    nc.finalize()
    nc._engine_load_estimate = dict(load)
    return nc


def _get_nc(sig):
    key = ("nc", sig)
    if key not in _CACHE:
        _CACHE[key] = _build_nc(sig)
    return _CACHE[key]


def _host_arrays(images, ys, xs, wins, perm, sig):
    """Build per-core input dicts for the signature."""
    f = np.float32
    slots = [(S, K, -(-S // 16)) for S, K in sig]
    TOTC = sum(s[2] for s in slots)

    p = np.arange(128, dtype=f)
    wneg = np.stack([-(p + 128.0 * k) for k in range(4)], axis=1).astype(f)

    in_maps = []
    for core in range(N_CORES):
        imgs = np.empty((PER_CORE, H, W, C), f)
        meta = np.full((8, 2 * NPAD), BAD, f)
        idxs = np.zeros((16, TOTC), np.int16)
        off = 0
        for j, (S, K, cols) in enumerate(slots):
            n = perm[core][j]
            imgs[j] = images[n]
            r0, Sn, cb0, Kn = wins[n]
            cb0p = min(cb0, 8 - K)
            meta[j, 0:CH] = ys[n] - f(r0)
            meta[j, NPAD:NPAD + CW] = xs[n] - f(64 * cb0p)
            for t in range(S):
                h = min(r0 + t, H - 1)
                idxs[t % 16, off + t // 16] = 24 * h + 3 * cb0p
            off += cols
        slabs = np.zeros((8 * 2 * NPAD, 32), f)
        for j in range(8):
            slabs[j * 2 * NPAD:(j + 1) * 2 * NPAD, 0] = meta[j]
        in_maps.append({
            "images": imgs,
            "meta": slabs,
            "idxs": np.tile(idxs, (8, 1)),
            "wneg": wneg,
        })
    return in_maps


def _ensure_device_platform():
    import jax
    try:
        if len([d for d in jax.devices() if d.platform != "cpu"]) >= N_CORES:
            return
    except Exception:
        pass
    import os
    os.environ.pop("JAX_PLATFORMS", None)
    try:
        jax.config.update("jax_platforms", None)
    except Exception:
        pass
    for clear in ("clear_backends",):
        try:
            getattr(jax, clear)()
            break
        except Exception:
            pass


def prepare(threshold, bboxes, images):
    """Host-side planning shared by kernel() and the sim test."""
    ys, xs = _host_coords(threshold, bboxes)
    wins = _windows(ys, xs)
    perm, sig = _plan(wins)
    images = np.ascontiguousarray(np.asarray(images, np.float32))
    in_maps = _host_arrays(images, ys, xs, wins, perm, sig)
    return in_maps, perm, sig


def assemble(results, perm):
    """results[core]["out"] [8, 3, 224, 224] -> full [64, 224, 224, 3]."""
    full = np.empty((N_FULL, CH, CW, C), np.float32)
    for core in range(N_CORES):
        o = np.asarray(results[core]["out"])
        o = np.transpose(o, (0, 2, 3, 1))
        for j in range(8):
            full[perm[core][j]] = o[j]
    return full


def kernel(threshold, bboxes, images):
    from concourse.bass_utils import run_bass_kernel_spmd

    _ensure_device_platform()
    in_maps, perm, sig = prepare(threshold, bboxes, images)
    nc = _get_nc(sig)
    _CACHE["nc"] = nc

    import os
    trace = bool(os.environ.get("CROP_TRACE"))
    if trace:
        try:
            import antenv.axon_hooks  # noqa: F401
        except ImportError:
            trace = False
    res = run_bass_kernel_spmd(nc, in_maps, list(range(N_CORES)), trace=trace)
    _CACHE["last_res"] = res
    return assemble(res.results, perm).astype(np.float32)
